# revision 2
# baseline (speedup 1.0000x reference)
"""Causal self-attention with RoPE on 8 Trainium2 NeuronCores.

Problem: B=4, T=2048, D=1024, H=16, Hd=64 (fp32).

Sharding: tensor-parallel over heads — 2 heads per core, all 4 batches on
every core. Each core computes q/k/v projections for its 2 heads, RoPE,
causal softmax(QK^T)V, and a row-sharded slice of out_proj; the host sums
the 8 partial outputs (the tensor-parallel all-reduce done at gather).

Numerics: float32r (tf32: fp32 with 10-bit mantissa) matmuls at full PE
rate; fp32 accumulation in PSUM; fp32 everywhere else.

Device dataflow per core, per batch:
  1. QKV projection: stationary = 128-column tiles of x^T (host-supplied),
     streaming = wqkv^T [128, 384] -> q|k|v natural [t, d] in PSUM.
  2. RoPE applied in natural layout (free-dim half swaps, host-built
     cos / +-sin tables), PE-transpose q,k -> qT,kT [128=2hx64, T].
  3. Scores^T per (head, tq-chunk 512, tk-tile 128): row-packed pair of
     matmuls (tile_position rows 0/64) -> sT psum [128 tk, 512 tq];
     exp via ScalarE with the 1/sqrt(Hd) scale folded in; causal diagonal
     tiles masked in-place by gpsimd.affine_select.
  4. AV: lhsT = v|ones [128, 65] -> out^T psum [65, 512]; row 64 = softmax
     denominators. Normalization: DVE copy of denom row -> reciprocal ->
     gpsimd.partition_broadcast -> DVE multiply at PSUM eviction.
  5. out_proj: lhsT = attn^T chunk [128, 128] (already transposed!),
     rhs = w_o slice^T [128, 1024] -> partial output, DMA to DRAM.
"""

import numpy as np
import concourse.bass as bass
import concourse.tile as tile
from concourse import bacc, mybir
from concourse.bass_utils import run_bass_kernel_spmd

F32 = mybir.dt.float32
F32R = mybir.dt.float32r
EXP = mybir.ActivationFunctionType.Exp

B, T, D, H, HD = 4, 2048, 1024, 16, 64
NC_ = 8                  # cores
HPC = H // NC_           # heads per core = 2
TT = T // 128            # 16 token tiles per batch
NCH = 4                  # tq chunks of 512
SCALE = 1.0 / np.sqrt(HD)


def to_tf32(a: np.ndarray) -> np.ndarray:
    u = np.ascontiguousarray(a, dtype=np.float32).view(np.uint32).astype(np.uint64)
    u = (u + 0xFFF + ((u >> 13) & 1)) & ~np.uint64(0x1FFF)
    return u.astype(np.uint32).view(np.float32)


def build_nc():
    nc = bacc.Bacc(None, target_bir_lowering=False)

    xt_d = nc.dram_tensor("xt", [B, 8, 128, T], F32R, kind="ExternalInput")
    wqkvt_d = nc.dram_tensor("wqkvt", [8, 128, 384], F32R, kind="ExternalInput")
    wot_d = nc.dram_tensor("wot", [128, 1024], F32R, kind="ExternalInput")
    cos_d = nc.dram_tensor("cosb", [128, TT, HD], F32, kind="ExternalInput")
    sin_d = nc.dram_tensor("sinb", [128, TT, HD], F32, kind="ExternalInput")
    ident_d = nc.dram_tensor("ident", [128, 128], F32, kind="ExternalInput")
    ones_d = nc.dram_tensor("vones", [128, TT, 2, 1], F32R, kind="ExternalInput")
    out_d = nc.dram_tensor("out", [B, TT, 128, D], F32, kind="ExternalOutput")

    with tile.TileContext(nc) as tc:
        with (
            tc.tile_pool(name="const", bufs=1) as const,
            tc.tile_pool(name="xtp", bufs=1) as xtp,
            tc.tile_pool(name="qkt", bufs=2) as qkt,
            tc.tile_pool(name="vp", bufs=2) as vp,
            tc.tile_pool(name="attnp", bufs=2) as attnp,
            tc.tile_pool(name="rope", bufs=3) as rope,
            tc.tile_pool(name="expp", bufs=6) as expp,
            tc.tile_pool(name="outst", bufs=3) as outst,
            tc.tile_pool(name="smallp", bufs=4) as smallp,
            tc.tile_pool(name="mmp", bufs=3, space="PSUM") as mmp,
            tc.tile_pool(name="scp", bufs=3, space="PSUM") as scp,
            tc.tile_pool(name="avp", bufs=1, space="PSUM") as avp,
        ):
            w_sb = const.tile([128, 8, 384], F32R)
            wo_sb = const.tile([128, 1024], F32R)
            cos_sb = const.tile([128, TT, HD], F32)
            sin_sb = const.tile([128, TT, HD], F32)
            ident_sb = const.tile([128, 128], F32)
            nc.sync.dma_start(w_sb[:], wqkvt_d[:].rearrange("c p n -> p c n"))
            nc.sync.dma_start(wo_sb[:], wot_d[:])
            nc.sync.dma_start(cos_sb[:], cos_d[:])
            nc.sync.dma_start(sin_sb[:], sin_d[:])
            nc.sync.dma_start(ident_sb[:], ident_d[:])

            for b in range(B):
                xt_sb = xtp.tile([128, 8, T], F32R, tag="xt")
                nc.sync.dma_start(xt_sb[:], xt_d[b].rearrange("c p t -> p c t"))

                qT_sb = qkt.tile([128, T], F32R, tag="qT")
                kT_sb = qkt.tile([128, T], F32R, tag="kT")
                v_sb = vp.tile([128, TT, 2, 65], F32R, tag="v")
                attn_sb = attnp.tile([128, T], F32R, tag="attn")
                nc.sync.dma_start(v_sb[:, :, :, 64:65], ones_d[:])

                # --- projection + rope + transpose, per 128-token tile ---
                for tt in range(TT):
                    pq = mmp.tile([128, 512], F32, tag="mm")
                    for dc in range(8):
                        nc.tensor.matmul(
                            pq[:, 0:384],
                            xt_sb[:, dc, tt * 128:(tt + 1) * 128],
                            w_sb[:, dc, :],
                            start=(dc == 0), stop=(dc == 7),
                        )
                    # rope: q|k cols 0:256; per 64-block: out = in*cos + swap(in)*sin'
                    qkc = rope.tile([128, 256], F32, tag="ropec")
                    qks = rope.tile([128, 256], F32, tag="ropes")
                    qkr = rope.tile([128, 256], F32, tag="roper")
                    pq4 = pq[:, 0:256].rearrange("p (b d) -> p b d", d=HD)
                    qkc4 = qkc[:].rearrange("p (b d) -> p b d", d=HD)
                    qks4 = qks[:].rearrange("p (b d) -> p b d", d=HD)
                    cos_bc = cos_sb[:, tt, None, :].to_broadcast((128, 4, HD))
                    sin_bc = sin_sb[:, tt, None, :].to_broadcast((128, 4, HD))
                    nc.vector.tensor_tensor(qkc4, pq4, cos_bc, mybir.AluOpType.mult)
                    # swapped halves: out[:, :, 0:32] = in[:, :, 32:64] * sin'[0:32]
                    nc.vector.tensor_tensor(
                        qks4[:, :, 0:32], pq4[:, :, 32:64],
                        sin_bc[:, :, 0:32], mybir.AluOpType.mult)
                    nc.vector.tensor_tensor(
                        qks4[:, :, 32:64], pq4[:, :, 0:32],
                        sin_bc[:, :, 32:64], mybir.AluOpType.mult)
                    nc.vector.tensor_tensor(qkr[:], qkc[:], qks[:],
                                            mybir.AluOpType.add)
                    # v eviction (rounds to f32r)
                    nc.vector.tensor_copy(
                        v_sb[:, tt, :, 0:64],
                        pq[:, 256:384].rearrange("p (h d) -> p h d", d=HD))
                    # PE transposes: q -> qT_sb, k -> kT_sb
                    for half, dst in ((0, qT_sb), (1, kT_sb)):
                        ptr = mmp.tile([128, 512], F32, tag="mm")
                        nc.tensor.transpose(
                            ptr[:, 0:128],
                            qkr[:, 128 * half:128 * half + 128],
                            ident_sb[:])
                        nc.vector.tensor_copy(
                            dst[:, tt * 128:(tt + 1) * 128], ptr[:, 0:128])

                # --- attention per tq chunk of 512 ---
                for c in range(NCH):
                    av0 = avp.tile([65, 512], F32, tag="av0")
                    av1 = avp.tile([65, 512], F32, tag="av1")
                    avs = (av0, av1)
                    njt = 4 * c + 4
                    for j in range(njt):
                        for h in (0, 1):
                            st = scp.tile([128, 512], F32, tag="st")
                            nc.tensor.matmul(
                                st[:],
                                kT_sb[64 * h:64 * h + 64, j * 128:(j + 1) * 128],
                                qT_sb[64 * h:64 * h + 64, c * 512:(c + 1) * 512],
                                start=True, stop=True,
                            )
                            ex = expp.tile([128, 512], F32R, tag="ex")
                            nc.scalar.activation(ex[:], st[:], EXP, scale=float(SCALE))
                            if j >= 4 * c:
                                nc.gpsimd.affine_select(
                                    out=ex[:], in_=ex[:],
                                    compare_op=mybir.AluOpType.is_ge,
                                    fill=0.0,
                                    base=512 * c - 128 * j,
                                    channel_multiplier=-1,
                                    pattern=[[1, 512]],
                                )
                            nc.tensor.matmul(
                                avs[h][:],
                                v_sb[:, j, h, :],
                                ex[:],
                                start=(j == 0), stop=(j == njt - 1),
                            )
                    for h in (0, 1):
                        den = smallp.tile([1, 512], F32, tag="den")
                        rec = smallp.tile([1, 512], F32, tag="rec")
                        bc = smallp.tile([64, 512], F32, tag="bc")
                        nc.vector.tensor_copy(den[0:1, :], avs[h][64:65, :])
                        nc.vector.reciprocal(rec[0:1, :], den[0:1, :])
                        nc.gpsimd.partition_broadcast(bc[:], rec[0:1, :])
                        nc.vector.tensor_tensor(
                            attn_sb[64 * h:64 * h + 64, c * 512:(c + 1) * 512],
                            avs[h][0:64, :], bc[:], mybir.AluOpType.mult)

                # --- out projection (partial over this core's 128 dims) ---
                for tt in range(TT):
                    for ch in (0, 1):
                        po = mmp.tile([128, 512], F32, tag="mm")
                        nc.tensor.matmul(
                            po[:],
                            attn_sb[:, tt * 128:(tt + 1) * 128],
                            wo_sb[:, ch * 512:(ch + 1) * 512],
                            start=True, stop=True,
                        )
                        ost = outst.tile([128, 512], F32, tag="ost")
                        nc.any.tensor_copy(ost[:], po[:])
                        nc.sync.dma_start(
                            out_d[b, tt, :, ch * 512:(ch + 1) * 512], ost[:])

    nc.compile()
    return nc


_NC_CACHE = None


def get_nc():
    global _NC_CACHE
    if _NC_CACHE is None:
        _NC_CACHE = build_nc()
    return _NC_CACHE


def prep_in_maps(x, cos, sin, w_q, w_k, w_v, w_o):
    """Host-side sharding: returns per-core input dicts."""
    x = np.asarray(x, np.float32)
    cos = np.asarray(cos, np.float32)
    sin = np.asarray(sin, np.float32)

    # x^T chunks: (B, 8, 128, T), tf32-rounded
    xt = to_tf32(np.ascontiguousarray(
        x.transpose(0, 2, 1)).reshape(B, 8, 128, T))

    # cos/sin in [t-partition, tile, d] layout; sin sign-flipped on low half
    cosb = np.ascontiguousarray(
        cos.reshape(TT, 128, HD).transpose(1, 0, 2))
    sinneg = sin.copy()
    sinneg[:, 0:HD // 2] *= -1.0
    sinb = np.ascontiguousarray(
        sinneg.reshape(TT, 128, HD).transpose(1, 0, 2))

    ident = np.eye(128, dtype=np.float32)
    vones = np.ones((128, TT, 2, 1), np.float32)

    in_maps = []
    for c in range(NC_):
        rows = slice(128 * c, 128 * (c + 1))  # 2 heads x 64 dims
        wqkv = np.concatenate([w_q[rows], w_k[rows], w_v[rows]], axis=0)  # (384, D)
        wqkvt = to_tf32(np.ascontiguousarray(wqkv.T).reshape(8, 128, 384))
        wot = to_tf32(np.ascontiguousarray(w_o[:, rows].T))  # (128, D)
        in_maps.append({
            "xt": xt, "wqkvt": wqkvt, "wot": wot,
            "cosb": cosb, "sinb": sinb, "ident": ident, "vones": vones,
        })
    return in_maps


def postprocess(results):
    out = np.zeros((B, TT, 128, D), np.float64)
    for r in results:
        out += r["out"].astype(np.float64)
    return out.reshape(B, T, D).astype(np.float32)


def kernel(x, cos, sin, w_q, w_k, w_v, w_o):
    nc = get_nc()
    in_maps = prep_in_maps(x, cos, sin, w_q, w_k, w_v, w_o)
    res = run_bass_kernel_spmd(nc, in_maps, core_ids=list(range(NC_)),
                               trace=False)
    return postprocess(res.results)
